# revision 6
# baseline (speedup 1.0000x reference)
"""Trainium2 Bass kernel for a binarized-weight BasicBlock (dense CNN).

Reference computation (all fp32):
    out = clip(bn2(conv3x3(quant(clip(bn1(conv3x3(quant(x), sign(w1))), -1, 1)),
                  sign(w2)) + x), -1, 1)
with quant(v) = round-half-up(v * 128) / 128 and bn in inference form.

Strategy:
  * Data-parallel: batch 32 is sharded 4 images per NeuronCore across 8 cores.
  * Channels (256) live on partitions as 2 blocks of 128.
  * conv3x3 = 18 accumulating matmuls per output tile (9 taps x 2 input
    channel blocks), fp16 operands / fp32 PSUM accumulation.  Activations are
    integers k = 128*quant(v) with |k| <= ~730 and weights are +-1, so every
    product and partial sum is exactly representable: the fp16 matmul path is
    bit-exact, and PSUM holds 128*conv exactly.
  * Weight-stationary chunking: the 18 weight tiles of an (conv, out-block)
    are each loaded once per chunk of 4 (resp. 3) row-groups; the 4 matmuls
    sharing a stationary tile accumulate into 4 different PSUM banks.  This
    cuts LDWEIGHTS traffic ~4x vs one load per matmul.
  * Activations are staged in zero-padded [128, blk, 58, 58] fp16 SBUF tiles;
    a conv matmul's moving operand is a strided [128, 8, 56] window, so no
    shift DMAs are needed.
  * quantize uses the +-1.5*2^23 magic add (RNE to integer).  The reference
    rounds half-up; RNE differs only on exact .5 ties, which have ~zero
    measure for these inputs (tolerance is 2e-2; observed mismatches 0).
  * BN is folded host-side to per-channel (inv, bias) fp32 pairs; the device
    applies psum*(inv/128) + bias with the same fp32 rounding sequence as the
    reference.
"""

import numpy as np

_N = 32          # full batch
_C = 256         # channels
_H = 56          # height
_W = 56          # width
_NCORES = 8
_EPS = 1e-5

_cache = {}


def _build(n_img, C, H, W, RG):
    """Build + compile the per-core Bass program (SPMD, one NEFF for all cores)."""
    from contextlib import ExitStack

    import concourse.tile as tile
    from concourse import bacc, mybir

    F32 = mybir.dt.float32
    F16 = mybir.dt.float16
    Alu = mybir.AluOpType
    Act = mybir.ActivationFunctionType

    MAGIC = float(3 << 22)  # 1.5 * 2**23: RNE-to-integer for |z| < 2**22

    nblk = C // 128
    ngrp = H // RG            # 7 row-groups of 8 rows
    HP, WP = H + 2, W + 2
    NW = 9 * nblk * nblk      # weight tiles per conv
    NWCHUNK = 9 * nblk        # weight tiles per (conv, ob) chunk
    CHUNKS = [(0, 4), (4, 3)]  # (first row-group, n row-groups) per psum chunk
    BANK = 512                # fp32 elems per PSUM bank
    RW = RG * W               # 448 cols per row-group

    nc = bacc.Bacc("TRN2", target_bir_lowering=False, debug=False,
                   num_devices=_NCORES)

    x_d = nc.dram_tensor("x", [n_img, C, H * W], F32, kind="ExternalInput")
    w_d = nc.dram_tensor("wq", [128, 2 * NW, 128], F16, kind="ExternalInput")
    c_d = nc.dram_tensor("coef", [128, nblk, 4], F32, kind="ExternalInput")
    o_d = nc.dram_tensor("out", [n_img, C, H * W], F32, kind="ExternalOutput")

    with tile.TileContext(nc) as tc, ExitStack() as ctx:
        const = ctx.enter_context(tc.tile_pool(name="const", bufs=1))
        xin = ctx.enter_context(tc.tile_pool(name="xin", bufs=2))
        pads = ctx.enter_context(tc.tile_pool(name="pads", bufs=1))
        q1s = ctx.enter_context(tc.tile_pool(name="q1s", bufs=3))
        e1s = ctx.enter_context(tc.tile_pool(name="e1s", bufs=2))
        e2s = ctx.enter_context(tc.tile_pool(name="e2s", bufs=2))
        psum = ctx.enter_context(tc.tile_pool(name="psum", bufs=1,
                                              space="PSUM"))

        # weight tiles grouped by (conv, ob) so the first-needed chunk's DMA
        # gates only the first matmuls, not the whole 2.4MB load; the first 4
        # tiles go in a mini-DMA so the warm-up matmuls can start early
        wt = const.tile([128, 2 * NW, 128], F16)
        nc.sync.dma_start(wt[:, 0:4, :], w_d.ap()[:, 0:4, :])

        # image 0 arrives in row-group pieces so the first conv chunk can
        # start as soon as row-groups 0-3 are quantized (the first 6 matmul
        # groups of chunk A only touch x rows 0-30)
        xi0 = x_d.ap()[0].rearrange("(b p) f -> p b f", p=128)
        xg0 = xin.tile([128, nblk, H * W], F32, tag="x", name="x0")

        def x0_dma(g0, gn):
            nc.sync.dma_start(xg0[:, :, g0 * RW:(g0 + gn) * RW],
                              xi0[:, :, g0 * RW:(g0 + gn) * RW])

        for g in range(4):
            x0_dma(g, 1)
        x0_dma(4, 1)
        ct = const.tile([128, nblk, 4], F32)
        nc.sync.dma_start(ct[:], c_d.ap())
        nc.sync.dma_start(wt[:, 4:NWCHUNK, :], w_d.ap()[:, 4:NWCHUNK, :])
        x0_dma(5, 2)
        for cv in range(2):
            for ob in range(nblk):
                if cv == 0 and ob == 0:
                    continue
                ch = (cv * nblk + ob) * NWCHUNK
                nc.sync.dma_start(wt[:, ch:ch + NWCHUNK, :],
                                  w_d.ap()[:, ch:ch + NWCHUNK, :])

        # dummy matmuls on the first mini-chunk: keeps the PE activity
        # monitor busy during the input fill so the real stream starts at
        # the full 2.4GHz clock (needs >3.4us of sustained PE activity).
        # The warm bank doubles as the last image's 1-row-group psum chunk.
        warm = psum.tile([128, 1, BANK], F32, tag="ps2", name="warm")
        for j in range(45):
            nc.tensor.matmul(warm[:, 0, 0:128], wt[:, 0, :], wt[:, j % 4, :],
                             start=True, stop=True)

        def conv_chunk(ps, pad, cv, ob, c0, cn):
            """Chunked conv: 18 weight tiles, each driving cn matmuls into
            cn different PSUM banks (stationary reuse)."""
            for ib in range(nblk):
                for tap in range(9):
                    dy, dx = tap // 3 - 1, tap % 3 - 1
                    widx = (cv * nblk + ob) * NWCHUNK + tap * nblk + ib
                    first = ib == 0 and tap == 0
                    last = ib == nblk - 1 and tap == 8
                    for j in range(cn):
                        r0 = (c0 + j) * RG
                        rhs = pad[:, ib, 1 + r0 + dy:1 + r0 + dy + RG,
                                  1 + dx:1 + dx + W]
                        nc.tensor.matmul(ps[:, j, 0:RW], wt[:, widx, :],
                                         rhs, start=first, stop=last)

        def zero_borders(pad):
            nc.vector.memset(pad[:, :, 0, :], 0.0)
            nc.vector.memset(pad[:, :, HP - 1, :], 0.0)
            nc.vector.memset(pad[:, :, 1:HP - 1, 0:1], 0.0)
            nc.vector.memset(pad[:, :, 1:HP - 1, WP - 1:WP], 0.0)

        for i in range(n_img):
            if i == 0:
                xg = xg0
            else:
                xg = xin.tile([128, nblk, H * W], F32, tag="x")
                xi = x_d.ap()[i].rearrange("(b p) f -> p b f", p=128)
                nc.sync.dma_start(xg[:, :, 0:4 * RW], xi[:, :, 0:4 * RW])
                nc.sync.dma_start(xg[:, :, 4 * RW:], xi[:, :, 4 * RW:])

            # quantize input into padded conv1 operand: k = RNE(128*x)
            pad1 = pads.tile([128, nblk, HP, WP], F16, tag="pad1")
            zero_borders(pad1)
            for g in range(ngrp):
                src = xg[:, :, g * RW:(g + 1) * RW]
                z = q1s.tile([128, nblk, RW], F32, tag="qz")
                nc.scalar.activation(z[:], src, Act.Copy, bias=0.0,
                                     scale=128.0)
                dst = pad1[:, :, 1 + g * RG:1 + (g + 1) * RG, 1:1 + W]
                nc.vector.tensor_scalar(
                    dst, z.rearrange("p b (h w) -> p b h w", w=W),
                    MAGIC, -MAGIC, Alu.add, Alu.add)

            # conv1 -> bn1 -> hardtanh -> quantize into padded conv2 operand
            pad2 = pads.tile([128, nblk, HP, WP], F16, tag="pad2")
            zero_borders(pad2)
            for ob in range(nblk):
                for ci, (c0, cn) in enumerate(CHUNKS):
                    ps = psum.tile([128, cn, BANK], F32,
                                   tag=f"ps{ci}", padded_shape=None)
                    conv_chunk(ps, pad1, 0, ob, c0, cn)
                    # y = inv1*psum + 128*bias1; clip to [-128,128]; RNE;
                    # write fp16 rows into pad2
                    psv = ps[:, :, 0:RW]
                    z = e1s.tile([128, cn, RW], F32, tag="z1",
                                 padded_shape=[128, 4, RW])
                    nc.scalar.activation(z[:], psv, Act.Identity,
                                         bias=ct[:, ob, 1:2],
                                         scale=ct[:, ob, 0:1])
                    cl = e1s.tile([128, cn, RW], F32, tag="c1",
                                  padded_shape=[128, 4, RW])
                    nc.vector.tensor_scalar(cl[:], z[:], 128.0, -128.0,
                                            Alu.min, Alu.max)
                    # the very last conv1 epilogue gates conv2's first
                    # matmuls on its first row-group: emit it row-group-wise
                    last_e1 = ob == nblk - 1 and ci == len(CHUNKS) - 1
                    pieces = [(j, 1) for j in range(cn)] if last_e1 \
                        else [(0, cn)]
                    for (p0, pn) in pieces:
                        dst = pad2[:, ob, 1 + (c0 + p0) * RG:
                                   1 + (c0 + p0 + pn) * RG, 1:1 + W]
                        nc.vector.tensor_scalar(
                            dst.rearrange("p (c h) w -> p c h w", h=RG),
                            cl[:, p0:p0 + pn, :].rearrange(
                                "p c (h w) -> p c h w", w=W),
                            MAGIC, -MAGIC, Alu.add, Alu.add)

            # conv2 -> +residual -> bn2 -> hardtanh -> out
            for ob in range(nblk):
                # the very last (image, ob) splits its second chunk 3+...
                # into 2+1 so the exposed kernel tail is a single
                # row-group's epilogue (the 1-rg chunk reuses the warm bank)
                final_ob = i == n_img - 1 and ob == nblk - 1
                c2chunks = [(0, 4, "ps0"), (4, 2, "ps1"), (6, 1, "ps2")] \
                    if final_ob else \
                    [(c0, cn, f"ps{ci}")
                     for ci, (c0, cn) in enumerate(CHUNKS)]
                for (c0, cn, ptag) in c2chunks:
                    ps = psum.tile([128, cn, BANK], F32, tag=ptag,
                                   name="ps2c")
                    conv_chunk(ps, pad2, 1, ob, c0, cn)
                    psv = ps[:, :, 0:RW]
                    pieces = [(0, cn)]
                    for (p0, pn) in pieces:
                        pv = psv[:, p0:p0 + pn, :]
                        res = xg[:, ob, (c0 + p0) * RW:(c0 + p0 + pn) * RW]
                        s = e2s.tile([128, pn, RW], F32, tag="s",
                                     padded_shape=[128, 4, RW])
                        nc.vector.scalar_tensor_tensor(
                            s[:], pv, 1.0 / 128.0,
                            res.rearrange("p (c f) -> p c f", f=RW),
                            Alu.mult, Alu.add)
                        bn = e2s.tile([128, pn, RW], F32, tag="bn2",
                                      padded_shape=[128, 4, RW])
                        nc.scalar.activation(bn[:], s[:], Act.Identity,
                                             bias=ct[:, ob, 3:4],
                                             scale=ct[:, ob, 2:3])
                        oc = e2s.tile([128, pn, RW], F32, tag="oc",
                                      padded_shape=[128, 4, RW])
                        nc.vector.tensor_scalar(oc[:], bn[:], 1.0, -1.0,
                                                Alu.min, Alu.max)
                        nc.sync.dma_start(
                            o_d.ap()[i, ob * 128:(ob + 1) * 128,
                                     (c0 + p0) * RW:(c0 + p0 + pn) * RW],
                            oc.rearrange("p c f -> p (c f)"))

    nc.compile()
    return nc


def _get_program(n_img, C, H, W, RG):
    key = (n_img, C, H, W, RG)
    if key not in _cache:
        _cache[key] = _build(n_img, C, H, W, RG)
    return _cache[key]


def _fold_bn(g, b, m, v):
    """Per-channel (inv, bias) in fp32, matching the reference's op sequence."""
    try:
        import jax

        with jax.default_device(jax.devices("cpu")[0]):
            inv = np.asarray(jax.jit(
                lambda g_, v_: g_ * jax.lax.rsqrt(v_ + _EPS), backend="cpu"
            )(g, v))
            bias = np.asarray(jax.jit(
                lambda b_, m_, i_: b_ - m_ * i_, backend="cpu"
            )(b, m, inv))
        return inv.astype(np.float32), bias.astype(np.float32)
    except Exception:
        inv = (g.astype(np.float32)
               * (np.float32(1.0) / np.sqrt(v.astype(np.float32)
                                            + np.float32(_EPS))))
        bias = b.astype(np.float32) - m.astype(np.float32) * inv
        return inv.astype(np.float32), bias.astype(np.float32)


def _prep_weights(w1, w2, C):
    """[128, 2*9*nblk*nblk, 128] fp16: lhsT tiles (i on partitions, o on free)."""
    nblk = C // 128
    tiles = np.empty((128, 2 * 9 * nblk * nblk, 128), np.float16)
    for cv, w in enumerate((w1, w2)):
        wq = np.where(w >= 0, np.float16(1.0), np.float16(-1.0))
        for ob in range(nblk):
            for tap in range(9):
                dy, dx = tap // 3, tap % 3
                for ib in range(nblk):
                    idx = (cv * nblk + ob) * 9 * nblk + tap * nblk + ib
                    blk = wq[ob * 128:(ob + 1) * 128,
                             ib * 128:(ib + 1) * 128, dy, dx]
                    tiles[:, idx, :] = blk.T
    return tiles


def _make_in_maps(x, w1, w2, g1, b1, m1, v1, g2, b2, m2, v2):
    n, C, H, W = x.shape
    n_img = n // _NCORES
    nblk = C // 128

    wq = _prep_weights(np.asarray(w1), np.asarray(w2), C)
    inv1, bias1 = _fold_bn(np.asarray(g1), np.asarray(b1),
                           np.asarray(m1), np.asarray(v1))
    inv2, bias2 = _fold_bn(np.asarray(g2), np.asarray(b2),
                           np.asarray(m2), np.asarray(v2))
    bias1z = np.float32(128.0) * bias1
    coef = np.empty((128, nblk, 4), np.float32)
    for blk in range(nblk):
        sl = slice(blk * 128, (blk + 1) * 128)
        coef[:, blk, 0] = inv1[sl]
        coef[:, blk, 1] = bias1z[sl]
        coef[:, blk, 2] = inv2[sl]
        coef[:, blk, 3] = bias2[sl]

    xr = np.ascontiguousarray(np.asarray(x).reshape(n, C, H * W),
                              dtype=np.float32)
    return [
        {"x": xr[i * n_img:(i + 1) * n_img], "wq": wq, "coef": coef}
        for i in range(_NCORES)
    ]


def _run(trace=False, **inputs):
    from concourse.bass_utils import run_bass_kernel_spmd

    n, C, H, W = inputs["x"].shape
    nc = _get_program(n // _NCORES, C, H, W, 8)
    in_maps = _make_in_maps(**inputs)
    res = run_bass_kernel_spmd(nc, in_maps, core_ids=list(range(_NCORES)),
                               trace=trace)
    out = np.concatenate([r["out"] for r in res.results], axis=0)
    return out.reshape(n, C, H, W), res


def kernel(x, w1, w2, g1, b1, m1, v1, g2, b2, m2, v2):
    out, _ = _run(x=x, w1=w1, w2=w2, g1=g1, b1=b1, m1=m1, v1=v1,
                  g2=g2, b2=b2, m2=m2, v2=v2)
    return out


# revision 12
# speedup vs baseline: 1.1933x; 1.1933x over previous
"""Trainium2 Bass kernel for a binarized-weight BasicBlock (dense CNN).

Reference computation (all fp32):
    out = clip(bn2(conv3x3(quant(clip(bn1(conv3x3(quant(x), sign(w1))), -1, 1)),
                  sign(w2)) + x), -1, 1)
with quant(v) = round-half-up(v * 128) / 128 and bn in inference form.

Strategy:
  * Data-parallel: batch 32 is sharded 4 images per NeuronCore across 8 cores.
  * Channels (256) live on partitions as 2 blocks of 128.
  * conv3x3 = 18 accumulating matmuls per output tile (9 taps x 2 input
    channel blocks), fp16 operands / fp32 PSUM accumulation.  Activations are
    integers k = 128*quant(v) with |k| <= ~730 and weights are +-1, so every
    product and partial sum is exactly representable: the fp16 matmul path is
    bit-exact, and PSUM holds 128*conv exactly.
  * Weight-stationary chunking: the 18 weight tiles of an (conv, out-block)
    are each loaded once per chunk of 4 (resp. 3) row-groups; the 4 matmuls
    sharing a stationary tile accumulate into 4 different PSUM banks.  This
    cuts LDWEIGHTS traffic ~4x vs one load per matmul.
  * Activations are staged in zero-padded [128, blk, 58, 58] fp16 SBUF tiles;
    a conv matmul's moving operand is a strided [128, 8, 56] window, so no
    shift DMAs are needed.
  * quantize uses the +-1.5*2^23 magic add (RNE to integer).  The reference
    rounds half-up; RNE differs only on exact .5 ties, which have ~zero
    measure for these inputs (tolerance is 2e-2; observed mismatches 0).
  * BN is folded host-side to per-channel (inv, bias) fp32 pairs; the device
    applies psum*(inv/128) + bias with the same fp32 rounding sequence as the
    reference.
"""

import numpy as np

_N = 32          # full batch
_C = 256         # channels
_H = 56          # height
_W = 56          # width
_NCORES = 8
_EPS = 1e-5

_cache = {}


def _build(n_img, C, H, W, RG):
    """Build + compile the per-core Bass program (SPMD, one NEFF for all cores)."""
    from contextlib import ExitStack

    import concourse.tile as tile
    from concourse import bacc, mybir

    F32 = mybir.dt.float32
    F16 = mybir.dt.float16
    Alu = mybir.AluOpType
    Act = mybir.ActivationFunctionType

    MAGIC = float(3 << 22)  # 1.5 * 2**23: RNE-to-integer for |z| < 2**22

    nblk = C // 128
    ngrp = H // RG            # 7 row-groups of 8 rows
    HP, WP = H + 2, W + 2
    NW = 9 * nblk * nblk      # weight tiles per conv
    NWCHUNK = 9 * nblk        # weight tiles per (conv, ob) chunk
    CHUNKS = [(0, 4), (4, 3)]  # (first row-group, n row-groups) per psum chunk
    BANK = 512                # fp32 elems per PSUM bank
    RW = RG * W               # 448 cols per row-group

    nc = bacc.Bacc("TRN2", target_bir_lowering=False, debug=False,
                   num_devices=_NCORES)

    x_d = nc.dram_tensor("x", [n_img, C, H * W], F32, kind="ExternalInput")
    w_d = nc.dram_tensor("wq", [128, 2 * NW, 128], F16, kind="ExternalInput")
    c_d = nc.dram_tensor("coef", [128, nblk, 4], F32, kind="ExternalInput")
    o_d = nc.dram_tensor("out", [n_img, C, H * W], F32, kind="ExternalOutput")

    with tile.TileContext(nc) as tc, ExitStack() as ctx:
        const = ctx.enter_context(tc.tile_pool(name="const", bufs=1))
        xin = ctx.enter_context(tc.tile_pool(name="xin", bufs=2))
        pads = ctx.enter_context(tc.tile_pool(name="pads", bufs=1))
        q1s = ctx.enter_context(tc.tile_pool(name="q1s", bufs=3))
        e1s = ctx.enter_context(tc.tile_pool(name="e1s", bufs=2))
        e2s = ctx.enter_context(tc.tile_pool(name="e2s", bufs=2))
        psum = ctx.enter_context(tc.tile_pool(name="psum", bufs=1,
                                              space="PSUM"))

        # weight tiles grouped by (conv, ob) so the first-needed chunk's DMA
        # gates only the first matmuls, not the whole 2.4MB load; the first 4
        # tiles go in a mini-DMA so the warm-up matmuls can start early
        # dummy matmuls on a memset tile: keeps the PE activity monitor busy
        # during the input fill so the real stream starts at the full 2.4GHz
        # clock (needs >3.4us of sustained PE activity).  Decoupled from any
        # DMA so it starts the moment the framework prologue ends, and sized
        # (32 x 512-col) to bridge until the first quantized rows are ready.
        # The warm bank doubles as the last image's 1-row-group psum chunk.
        wsrc = const.tile([128, BANK], F16)
        nc.vector.memset(wsrc[:], 0.0)
        warm = psum.tile([128, 1, BANK], F32, tag="ps2", name="warm")
        for j in range(32):
            nc.tensor.matmul(warm[:, 0, :], wsrc[:, 0:128], wsrc[:],
                             start=True, stop=True)

        # weight tiles are ib-major within a (conv, ob) chunk, so the first
        # 9-tile mini-DMA covers all of ib=0's taps (the first 9 matmul
        # groups); image 0 follows in row-group pieces so the first conv
        # chunk can start as soon as row-groups 0-3 are quantized (the
        # first matmul groups only touch x rows 0-30)
        xi0 = x_d.ap()[0].rearrange("(b p) f -> p b f", p=128)
        xg0 = xin.tile([128, nblk, H * W], F32, tag="x", name="x0")

        wt = const.tile([128, 2 * NW, 128], F16)
        nc.sync.dma_start(wt[:, 0:9, :], w_d.ap()[:, 0:9, :])

        def x0_dma(g0, gn):
            nc.sync.dma_start(xg0[:, :, g0 * RW:(g0 + gn) * RW],
                              xi0[:, :, g0 * RW:(g0 + gn) * RW])

        for g in range(4):
            x0_dma(g, 1)
        nc.sync.dma_start(wt[:, 9:NWCHUNK, :], w_d.ap()[:, 9:NWCHUNK, :])
        x0_dma(4, 1)
        ct = const.tile([128, nblk, 4], F32)
        nc.sync.dma_start(ct[:], c_d.ap())
        x0_dma(5, 2)
        for cv in range(2):
            for ob in range(nblk):
                if cv == 0 and ob == 0:
                    continue
                ch = (cv * nblk + ob) * NWCHUNK
                nc.sync.dma_start(wt[:, ch:ch + NWCHUNK, :],
                                  w_d.ap()[:, ch:ch + NWCHUNK, :])

        def conv_chunk(ps, pad, cv, ob, c0, cn):
            """Chunked conv: 18 weight tiles, each driving cn matmuls into
            cn different PSUM banks (stationary reuse)."""
            for ib in range(nblk):
                for tap in range(9):
                    dy, dx = tap // 3 - 1, tap % 3 - 1
                    widx = (cv * nblk + ob) * NWCHUNK + ib * 9 + tap
                    first = ib == 0 and tap == 0
                    last = ib == nblk - 1 and tap == 8
                    for j in range(cn):
                        r0 = (c0 + j) * RG
                        rhs = pad[:, ib, 1 + r0 + dy:1 + r0 + dy + RG,
                                  1 + dx:1 + dx + W]
                        nc.tensor.matmul(ps[:, j, 0:RW], wt[:, widx, :],
                                         rhs, start=first, stop=last)

        def zero_borders(pad):
            nc.vector.memset(pad[:, :, 0, :], 0.0)
            nc.vector.memset(pad[:, :, HP - 1, :], 0.0)
            nc.vector.memset(pad[:, :, 1:HP - 1, 0:1], 0.0)
            nc.vector.memset(pad[:, :, 1:HP - 1, WP - 1:WP], 0.0)

        for i in range(n_img):
            if i == 0:
                xg = xg0
            else:
                xg = xin.tile([128, nblk, H * W], F32, tag="x")
                xi = x_d.ap()[i].rearrange("(b p) f -> p b f", p=128)
                nc.sync.dma_start(xg[:, :, 0:4 * RW], xi[:, :, 0:4 * RW])
                nc.sync.dma_start(xg[:, :, 4 * RW:], xi[:, :, 4 * RW:])

            # quantize input into padded conv1 operand: k = RNE(128*x)
            pad1 = pads.tile([128, nblk, HP, WP], F16, tag="pad1")
            zero_borders(pad1)
            for g in range(ngrp):
                src = xg[:, :, g * RW:(g + 1) * RW]
                z = q1s.tile([128, nblk, RW], F32, tag="qz")
                nc.scalar.activation(z[:], src, Act.Copy, bias=0.0,
                                     scale=128.0)
                dst = pad1[:, :, 1 + g * RG:1 + (g + 1) * RG, 1:1 + W]
                nc.vector.tensor_scalar(
                    dst, z.rearrange("p b (h w) -> p b h w", w=W),
                    MAGIC, -MAGIC, Alu.add, Alu.add)

            # conv1 -> bn1 -> hardtanh -> quantize into padded conv2 operand
            pad2 = pads.tile([128, nblk, HP, WP], F16, tag="pad2")
            zero_borders(pad2)
            for ob in range(nblk):
                for ci, (c0, cn) in enumerate(CHUNKS):
                    ps = psum.tile([128, cn, BANK], F32,
                                   tag=f"ps{ci}", padded_shape=None)
                    conv_chunk(ps, pad1, 0, ob, c0, cn)
                    # y = inv1*psum + 128*bias1; clip to [-128,128]; RNE;
                    # write fp16 rows into pad2
                    psv = ps[:, :, 0:RW]
                    z = e1s.tile([128, cn, RW], F32, tag="z1",
                                 padded_shape=[128, 4, RW])
                    nc.scalar.activation(z[:], psv, Act.Identity,
                                         bias=ct[:, ob, 1:2],
                                         scale=ct[:, ob, 0:1])
                    cl = e1s.tile([128, cn, RW], F32, tag="c1",
                                  padded_shape=[128, 4, RW])
                    nc.vector.tensor_scalar(cl[:], z[:], 128.0, -128.0,
                                            Alu.min, Alu.max)
                    # the very last conv1 epilogue gates conv2's first
                    # matmuls on its first row-group: emit it row-group-wise
                    last_e1 = ob == nblk - 1 and ci == len(CHUNKS) - 1
                    pieces = [(j, 1) for j in range(cn)] if last_e1 \
                        else [(0, cn)]
                    for (p0, pn) in pieces:
                        dst = pad2[:, ob, 1 + (c0 + p0) * RG:
                                   1 + (c0 + p0 + pn) * RG, 1:1 + W]
                        nc.vector.tensor_scalar(
                            dst.rearrange("p (c h) w -> p c h w", h=RG),
                            cl[:, p0:p0 + pn, :].rearrange(
                                "p c (h w) -> p c h w", w=W),
                            MAGIC, -MAGIC, Alu.add, Alu.add)

            # conv2 -> +residual -> bn2 -> hardtanh -> out
            for ob in range(nblk):
                # the very last (image, ob) splits its second chunk 3+...
                # into 2+1 so the exposed kernel tail is a single
                # row-group's epilogue (the 1-rg chunk reuses the warm bank)
                final_ob = i == n_img - 1 and ob == nblk - 1
                c2chunks = [(0, 4, "ps0"), (4, 2, "ps1"), (6, 1, "ps2")] \
                    if final_ob else \
                    [(c0, cn, f"ps{ci}")
                     for ci, (c0, cn) in enumerate(CHUNKS)]
                for (c0, cn, ptag) in c2chunks:
                    ps = psum.tile([128, cn, BANK], F32, tag=ptag,
                                   name="ps2c")
                    conv_chunk(ps, pad2, 1, ob, c0, cn)
                    # the very last chunk's epilogue is the exposed kernel
                    # tail: run it in column halves so ACT/DVE/DMA pipeline
                    if final_ob and ptag == "ps2":
                        pieces = [(0, RW // 2), (RW // 2, RW // 2)]
                    else:
                        pieces = [(0, cn * RW)]
                    for (h0, hn) in pieces:
                        # strided view: cn banks each contributing RW cols
                        cs, ce = h0 // RW, (h0 + hn - 1) // RW + 1
                        pn = ce - cs
                        o0 = h0 - cs * RW
                        pvw = ps[:, cs:ce, o0:o0 + min(hn, RW)]
                        res = xg[:, ob, c0 * RW + h0:c0 * RW + h0 + hn]
                        s = e2s.tile([128, pn, min(hn, RW)], F32, tag="s",
                                     padded_shape=[128, 4, RW])
                        nc.vector.scalar_tensor_tensor(
                            s[:], pvw, 1.0 / 128.0,
                            res.rearrange("p (c f) -> p c f", c=pn),
                            Alu.mult, Alu.add)
                        bn = e2s.tile([128, pn, min(hn, RW)], F32,
                                      tag="bn2", padded_shape=[128, 4, RW])
                        nc.scalar.activation(bn[:], s[:], Act.Identity,
                                             bias=ct[:, ob, 3:4],
                                             scale=ct[:, ob, 2:3])
                        oc = e2s.tile([128, pn, min(hn, RW)], F32,
                                      tag="oc", padded_shape=[128, 4, RW])
                        nc.vector.tensor_scalar(oc[:], bn[:], 1.0, -1.0,
                                                Alu.min, Alu.max)
                        nc.sync.dma_start(
                            o_d.ap()[i, ob * 128:(ob + 1) * 128,
                                     c0 * RW + h0:c0 * RW + h0 + hn],
                            oc.rearrange("p c f -> p (c f)"))

    nc.compile()
    return nc


def _get_program(n_img, C, H, W, RG):
    key = (n_img, C, H, W, RG)
    if key not in _cache:
        _cache[key] = _build(n_img, C, H, W, RG)
    return _cache[key]


def _fold_bn(g, b, m, v):
    """Per-channel (inv, bias) in fp32, matching the reference's op sequence."""
    try:
        import jax

        with jax.default_device(jax.devices("cpu")[0]):
            inv = np.asarray(jax.jit(
                lambda g_, v_: g_ * jax.lax.rsqrt(v_ + _EPS), backend="cpu"
            )(g, v))
            bias = np.asarray(jax.jit(
                lambda b_, m_, i_: b_ - m_ * i_, backend="cpu"
            )(b, m, inv))
        return inv.astype(np.float32), bias.astype(np.float32)
    except Exception:
        inv = (g.astype(np.float32)
               * (np.float32(1.0) / np.sqrt(v.astype(np.float32)
                                            + np.float32(_EPS))))
        bias = b.astype(np.float32) - m.astype(np.float32) * inv
        return inv.astype(np.float32), bias.astype(np.float32)


def _prep_weights(w1, w2, C):
    """[128, 2*9*nblk*nblk, 128] fp16: lhsT tiles (i on partitions, o on free)."""
    nblk = C // 128
    tiles = np.empty((128, 2 * 9 * nblk * nblk, 128), np.float16)
    for cv, w in enumerate((w1, w2)):
        wq = np.where(w >= 0, np.float16(1.0), np.float16(-1.0))
        for ob in range(nblk):
            for ib in range(nblk):
                for tap in range(9):
                    dy, dx = tap // 3, tap % 3
                    idx = (cv * nblk + ob) * 9 * nblk + ib * 9 + tap
                    blk = wq[ob * 128:(ob + 1) * 128,
                             ib * 128:(ib + 1) * 128, dy, dx]
                    tiles[:, idx, :] = blk.T
    return tiles


def _make_in_maps(x, w1, w2, g1, b1, m1, v1, g2, b2, m2, v2):
    n, C, H, W = x.shape
    n_img = n // _NCORES
    nblk = C // 128

    wq = _prep_weights(np.asarray(w1), np.asarray(w2), C)
    inv1, bias1 = _fold_bn(np.asarray(g1), np.asarray(b1),
                           np.asarray(m1), np.asarray(v1))
    inv2, bias2 = _fold_bn(np.asarray(g2), np.asarray(b2),
                           np.asarray(m2), np.asarray(v2))
    bias1z = np.float32(128.0) * bias1
    coef = np.empty((128, nblk, 4), np.float32)
    for blk in range(nblk):
        sl = slice(blk * 128, (blk + 1) * 128)
        coef[:, blk, 0] = inv1[sl]
        coef[:, blk, 1] = bias1z[sl]
        coef[:, blk, 2] = inv2[sl]
        coef[:, blk, 3] = bias2[sl]

    xr = np.ascontiguousarray(np.asarray(x).reshape(n, C, H * W),
                              dtype=np.float32)
    return [
        {"x": xr[i * n_img:(i + 1) * n_img], "wq": wq, "coef": coef}
        for i in range(_NCORES)
    ]


def _run(trace=False, **inputs):
    from concourse.bass_utils import run_bass_kernel_spmd

    n, C, H, W = inputs["x"].shape
    nc = _get_program(n // _NCORES, C, H, W, 8)
    in_maps = _make_in_maps(**inputs)
    res = run_bass_kernel_spmd(nc, in_maps, core_ids=list(range(_NCORES)),
                               trace=trace)
    out = np.concatenate([r["out"] for r in res.results], axis=0)
    return out.reshape(n, C, H, W), res


def kernel(x, w1, w2, g1, b1, m1, v1, g2, b2, m2, v2):
    out, _ = _run(x=x, w1=w1, w2=w2, g1=g1, b1=b1, m1=m1, v1=v1,
                  g2=g2, b2=b2, m2=m2, v2=v2)
    return out


# revision 19
# speedup vs baseline: 1.1968x; 1.0029x over previous
"""Trainium2 Bass kernel for a binarized-weight BasicBlock (dense CNN).

Reference computation (all fp32):
    out = clip(bn2(conv3x3(quant(clip(bn1(conv3x3(quant(x), sign(w1))), -1, 1)),
                  sign(w2)) + x), -1, 1)
with quant(v) = round-half-up(v * 128) / 128 and bn in inference form.

Strategy:
  * Data-parallel: batch 32 is sharded 4 images per NeuronCore across 8 cores.
  * Channels (256) live on partitions as 2 blocks of 128.
  * conv3x3 = 18 accumulating matmuls per output tile (9 taps x 2 input
    channel blocks), fp16 operands / fp32 PSUM accumulation.  Activations are
    integers k = 128*quant(v) with |k| <= ~730 and weights are +-1, so every
    product and partial sum is exactly representable: the fp16 matmul path is
    bit-exact, and PSUM holds 128*conv exactly.
  * Weight-stationary chunking: the 18 weight tiles of an (conv, out-block)
    are each loaded once per chunk of 4 (resp. 3) row-groups; the 4 matmuls
    sharing a stationary tile accumulate into 4 different PSUM banks.  This
    cuts LDWEIGHTS traffic ~4x vs one load per matmul.
  * Activations are staged in zero-padded [128, blk, 58, 58] fp16 SBUF tiles;
    a conv matmul's moving operand is a strided [128, 8, 56] window, so no
    shift DMAs are needed.
  * quantize uses the +-1.5*2^23 magic add (RNE to integer).  The reference
    rounds half-up; RNE differs only on exact .5 ties, which have ~zero
    measure for these inputs (tolerance is 2e-2; observed mismatches 0).
  * BN is folded host-side to per-channel (inv, bias) fp32 pairs; the device
    applies psum*(inv/128) + bias with the same fp32 rounding sequence as the
    reference.
"""

import numpy as np

_N = 32          # full batch
_C = 256         # channels
_H = 56          # height
_W = 56          # width
_NCORES = 8
_EPS = 1e-5

_cache = {}


def _build(n_img, C, H, W, RG):
    """Build + compile the per-core Bass program (SPMD, one NEFF for all cores)."""
    from contextlib import ExitStack

    import concourse.tile as tile
    from concourse import bacc, mybir

    F32 = mybir.dt.float32
    F16 = mybir.dt.float16
    Alu = mybir.AluOpType
    Act = mybir.ActivationFunctionType

    MAGIC = float(3 << 22)  # 1.5 * 2**23: RNE-to-integer for |z| < 2**22

    nblk = C // 128
    ngrp = H // RG            # 7 row-groups of 8 rows
    HP, WP = H + 2, W + 2
    NW = 9 * nblk * nblk      # weight tiles per conv
    NWCHUNK = 9 * nblk        # weight tiles per (conv, ob) chunk
    CHUNKS = [(0, 4), (4, 3)]  # (first row-group, n row-groups) per psum chunk
    BANK = 512                # fp32 elems per PSUM bank
    RW = RG * W               # 448 cols per row-group

    nc = bacc.Bacc("TRN2", target_bir_lowering=False, debug=False,
                   num_devices=_NCORES)

    x_d = nc.dram_tensor("x", [n_img, C, H * W], F32, kind="ExternalInput")
    w_d = nc.dram_tensor("wq", [128, 2 * NW, 128], F16, kind="ExternalInput")
    c_d = nc.dram_tensor("coef", [128, nblk, 4], F32, kind="ExternalInput")
    o_d = nc.dram_tensor("out", [n_img, C, H * W], F32, kind="ExternalOutput")

    with tile.TileContext(nc) as tc, ExitStack() as ctx:
        const = ctx.enter_context(tc.tile_pool(name="const", bufs=1))
        xin = ctx.enter_context(tc.tile_pool(name="xin", bufs=2))
        pads = ctx.enter_context(tc.tile_pool(name="pads", bufs=1))
        q1s = ctx.enter_context(tc.tile_pool(name="q1s", bufs=3))
        e1s = ctx.enter_context(tc.tile_pool(name="e1s", bufs=2))
        e2s = ctx.enter_context(tc.tile_pool(name="e2s", bufs=2))
        psum = ctx.enter_context(tc.tile_pool(name="psum", bufs=1,
                                              space="PSUM"))

        # weight tiles grouped by (conv, ob) so the first-needed chunk's DMA
        # gates only the first matmuls, not the whole 2.4MB load; the first 4
        # tiles go in a mini-DMA so the warm-up matmuls can start early
        # dummy matmuls on a memset tile: keeps the PE activity monitor busy
        # during the input fill so the real stream starts at the full 2.4GHz
        # clock (needs >3.4us of sustained PE activity).  Decoupled from any
        # DMA so it starts the moment the framework prologue ends, and sized
        # (32 x 512-col) to bridge until the first quantized rows are ready.
        # The warm bank doubles as the last image's 1-row-group psum chunk.
        wsrc = const.tile([128, BANK], F16)
        nc.vector.memset(wsrc[:], 0.0)
        warm = psum.tile([128, 1, BANK], F32, tag="ps2", name="warm")
        for j in range(9):
            nc.tensor.matmul(warm[:, 0, :], wsrc[:, 0:128], wsrc[:],
                             start=True, stop=True)

        # DMA priority order: conv1-ob0's full weight chunk, then image 0
        # in row pieces (2-row pieces for rows 0-9, which gate the first
        # 1-row-group conv chunk), then the rest.  The sync engine issues
        # serially at ~700ns/DMA and the wire runs at ~370GB/s, so this
        # order sets the critical path to the first real matmul (~12us).
        xi0 = x_d.ap()[0].rearrange("(b p) f -> p b f", p=128)
        xg0 = xin.tile([128, nblk, H * W], F32, tag="x", name="x0")

        wt = const.tile([128, 2 * NW, 128], F16)
        nc.sync.dma_start(wt[:, 0:NWCHUNK, :], w_d.ap()[:, 0:NWCHUNK, :])

        X0_PIECES = [(0, 2), (2, 2), (4, 2), (6, 2), (8, 2), (10, 6),
                     (16, 8), (24, 8), (32, 8), (40, 8), (48, 8)]

        def x0_dma(r0, rn):
            nc.sync.dma_start(xg0[:, :, r0 * W:(r0 + rn) * W],
                              xi0[:, :, r0 * W:(r0 + rn) * W])

        for (r0, rn) in X0_PIECES[:6]:
            x0_dma(r0, rn)
        x0_dma(16, 8)
        x0_dma(24, 8)
        x0_dma(32, 8)
        x0_dma(40, 16)
        ch1 = 1 * NWCHUNK  # conv1-ob1 weights
        nc.sync.dma_start(wt[:, ch1:ch1 + NWCHUNK, :],
                          w_d.ap()[:, ch1:ch1 + NWCHUNK, :])
        ct = const.tile([128, nblk, 4], F32)
        nc.sync.dma_start(ct[:], c_d.ap())
        for ch in (2 * NWCHUNK, 3 * NWCHUNK):
            nc.sync.dma_start(wt[:, ch:ch + NWCHUNK, :],
                              w_d.ap()[:, ch:ch + NWCHUNK, :])

        def conv_chunk(ps, pad, cv, ob, c0, cn):
            """Chunked conv: 18 weight tiles, each driving matmuls over
            pairs of row-groups (896-col moving operand, 2 PSUM banks per
            instruction) with stationary reuse."""
            jps = [(j, 1) for j in range(cn)]
            for ib in range(nblk):
                for tap in range(9):
                    dy, dx = tap // 3 - 1, tap % 3 - 1
                    widx = (cv * nblk + ob) * NWCHUNK + ib * 9 + tap
                    first = ib == 0 and tap == 0
                    last = ib == nblk - 1 and tap == 8
                    for (j0, jn) in jps:
                        r0 = (c0 + j0) * RG
                        rhs = pad[:, ib, 1 + r0 + dy:1 + r0 + dy + jn * RG,
                                  1 + dx:1 + dx + W]
                        nc.tensor.matmul(ps[:, j0:j0 + jn, 0:RW],
                                         wt[:, widx, :],
                                         rhs, start=first, stop=last)

        def zero_borders(pad):
            nc.vector.memset(pad[:, :, 0, :], 0.0)
            nc.vector.memset(pad[:, :, HP - 1, :], 0.0)
            nc.vector.memset(pad[:, :, 1:HP - 1, 0:1], 0.0)
            nc.vector.memset(pad[:, :, 1:HP - 1, WP - 1:WP], 0.0)

        for i in range(n_img):
            if i == 0:
                xg = xg0
            else:
                xg = xin.tile([128, nblk, H * W], F32, tag="x")
                xi = x_d.ap()[i].rearrange("(b p) f -> p b f", p=128)
                nc.sync.dma_start(xg[:, :, 0:4 * RW], xi[:, :, 0:4 * RW])
                nc.sync.dma_start(xg[:, :, 4 * RW:], xi[:, :, 4 * RW:])

            # quantize input into padded conv1 operand: k = RNE(128*x)
            pad1 = pads.tile([128, nblk, HP, WP], F16, tag="pad1")
            zero_borders(pad1)
            qpieces = X0_PIECES if i == 0 else \
                [(g * RG, RG) for g in range(ngrp)]
            for (r0, rn) in qpieces:
                src = xg[:, :, r0 * W:(r0 + rn) * W]
                z = q1s.tile([128, nblk, rn * W], F32, tag="qz",
                             padded_shape=[128, nblk, RW])
                nc.scalar.activation(z[:], src, Act.Copy, bias=0.0,
                                     scale=128.0)
                dst = pad1[:, :, 1 + r0:1 + r0 + rn, 1:1 + W]
                nc.vector.tensor_scalar(
                    dst, z.rearrange("p b (h w) -> p b h w", w=W),
                    MAGIC, -MAGIC, Alu.add, Alu.add)

            # conv1 -> bn1 -> hardtanh -> quantize into padded conv2 operand
            # image 0, ob 0 starts with a 1-row-group chunk so the first
            # matmuls only gate on x rows 0-9
            pad2 = pads.tile([128, nblk, HP, WP], F16, tag="pad2")
            zero_borders(pad2)
            for ob in range(nblk):
                if i == 0 and ob == 0:
                    c1chunks = [(0, 1, "ps2"), (1, 3, "ps1"), (4, 3, "ps0")]
                elif i == 0:
                    c1chunks = [(0, 3, "ps1"), (3, 4, "ps0")]
                else:
                    c1chunks = [(c0, cn, f"ps{ci}")
                                for ci, (c0, cn) in enumerate(CHUNKS)]
                for ci, (c0, cn, ptag) in enumerate(c1chunks):
                    ps = psum.tile([128, cn, BANK], F32,
                                   tag=ptag, name="ps1c")
                    conv_chunk(ps, pad1, 0, ob, c0, cn)
                    # y = inv1*psum + 128*bias1; clip to [-128,128]; RNE;
                    # write fp16 rows into pad2
                    psv = ps[:, :, 0:RW]
                    z = e1s.tile([128, cn, RW], F32, tag="z1",
                                 padded_shape=[128, 4, RW])
                    nc.scalar.activation(z[:], psv, Act.Identity,
                                         bias=ct[:, ob, 1:2],
                                         scale=ct[:, ob, 0:1])
                    cl = e1s.tile([128, cn, RW], F32, tag="c1",
                                  padded_shape=[128, 4, RW])
                    nc.vector.tensor_scalar(cl[:], z[:], 128.0, -128.0,
                                            Alu.min, Alu.max)
                    # the very last conv1 epilogue gates conv2's first
                    # matmuls on its first row-group: emit it row-group-wise
                    last_e1 = ob == nblk - 1 and ci == len(c1chunks) - 1
                    pieces = [(j, 1) for j in range(cn)] if last_e1 \
                        else [(0, cn)]
                    for (p0, pn) in pieces:
                        dst = pad2[:, ob, 1 + (c0 + p0) * RG:
                                   1 + (c0 + p0 + pn) * RG, 1:1 + W]
                        nc.vector.tensor_scalar(
                            dst.rearrange("p (c h) w -> p c h w", h=RG),
                            cl[:, p0:p0 + pn, :].rearrange(
                                "p c (h w) -> p c h w", w=W),
                            MAGIC, -MAGIC, Alu.add, Alu.add)

            # conv2 -> +residual -> bn2 -> hardtanh -> out
            for ob in range(nblk):
                # the very last (image, ob) splits its second chunk 3+...
                # into 2+1 so the exposed kernel tail is a single
                # row-group's epilogue (the 1-rg chunk reuses the warm bank)
                final_ob = i == n_img - 1 and ob == nblk - 1
                c2chunks = [(0, 4, "ps0"), (4, 2, "ps1"), (6, 1, "ps2")] \
                    if final_ob else \
                    [(c0, cn, f"ps{ci}")
                     for ci, (c0, cn) in enumerate(CHUNKS)]
                for (c0, cn, ptag) in c2chunks:
                    ps = psum.tile([128, cn, BANK], F32, tag=ptag,
                                   name="ps2c")
                    conv_chunk(ps, pad2, 1, ob, c0, cn)
                    # the very last chunk's epilogue is the exposed kernel
                    # tail: run it in column halves so ACT/DVE/DMA pipeline
                    if final_ob and ptag == "ps2":
                        pieces = [(0, RW // 2), (RW // 2, RW // 2)]
                    else:
                        pieces = [(0, cn * RW)]
                    for (h0, hn) in pieces:
                        # strided view: cn banks each contributing RW cols
                        cs, ce = h0 // RW, (h0 + hn - 1) // RW + 1
                        pn = ce - cs
                        o0 = h0 - cs * RW
                        pvw = ps[:, cs:ce, o0:o0 + min(hn, RW)]
                        res = xg[:, ob, c0 * RW + h0:c0 * RW + h0 + hn]
                        s = e2s.tile([128, pn, min(hn, RW)], F32, tag="s",
                                     padded_shape=[128, 4, RW])
                        nc.vector.scalar_tensor_tensor(
                            s[:], pvw, 1.0 / 128.0,
                            res.rearrange("p (c f) -> p c f", c=pn),
                            Alu.mult, Alu.add)
                        bn = e2s.tile([128, pn, min(hn, RW)], F32,
                                      tag="bn2", padded_shape=[128, 4, RW])
                        nc.scalar.activation(bn[:], s[:], Act.Identity,
                                             bias=ct[:, ob, 3:4],
                                             scale=ct[:, ob, 2:3])
                        oc = e2s.tile([128, pn, min(hn, RW)], F32,
                                      tag="oc", padded_shape=[128, 4, RW])
                        nc.vector.tensor_scalar(oc[:], bn[:], 1.0, -1.0,
                                                Alu.min, Alu.max)
                        nc.sync.dma_start(
                            o_d.ap()[i, ob * 128:(ob + 1) * 128,
                                     c0 * RW + h0:c0 * RW + h0 + hn],
                            oc.rearrange("p c f -> p (c f)"))

    nc.compile()
    return nc


def _get_program(n_img, C, H, W, RG):
    key = (n_img, C, H, W, RG)
    if key not in _cache:
        _cache[key] = _build(n_img, C, H, W, RG)
    return _cache[key]


def _fold_bn(g, b, m, v):
    """Per-channel (inv, bias) in fp32, matching the reference's op sequence."""
    try:
        import jax

        with jax.default_device(jax.devices("cpu")[0]):
            inv = np.asarray(jax.jit(
                lambda g_, v_: g_ * jax.lax.rsqrt(v_ + _EPS), backend="cpu"
            )(g, v))
            bias = np.asarray(jax.jit(
                lambda b_, m_, i_: b_ - m_ * i_, backend="cpu"
            )(b, m, inv))
        return inv.astype(np.float32), bias.astype(np.float32)
    except Exception:
        inv = (g.astype(np.float32)
               * (np.float32(1.0) / np.sqrt(v.astype(np.float32)
                                            + np.float32(_EPS))))
        bias = b.astype(np.float32) - m.astype(np.float32) * inv
        return inv.astype(np.float32), bias.astype(np.float32)


def _prep_weights(w1, w2, C):
    """[128, 2*9*nblk*nblk, 128] fp16: lhsT tiles (i on partitions, o on free)."""
    nblk = C // 128
    tiles = np.empty((128, 2 * 9 * nblk * nblk, 128), np.float16)
    for cv, w in enumerate((w1, w2)):
        wq = np.where(w >= 0, np.float16(1.0), np.float16(-1.0))
        for ob in range(nblk):
            for ib in range(nblk):
                for tap in range(9):
                    dy, dx = tap // 3, tap % 3
                    idx = (cv * nblk + ob) * 9 * nblk + ib * 9 + tap
                    blk = wq[ob * 128:(ob + 1) * 128,
                             ib * 128:(ib + 1) * 128, dy, dx]
                    tiles[:, idx, :] = blk.T
    return tiles


def _make_in_maps(x, w1, w2, g1, b1, m1, v1, g2, b2, m2, v2):
    n, C, H, W = x.shape
    n_img = n // _NCORES
    nblk = C // 128

    wq = _prep_weights(np.asarray(w1), np.asarray(w2), C)
    inv1, bias1 = _fold_bn(np.asarray(g1), np.asarray(b1),
                           np.asarray(m1), np.asarray(v1))
    inv2, bias2 = _fold_bn(np.asarray(g2), np.asarray(b2),
                           np.asarray(m2), np.asarray(v2))
    bias1z = np.float32(128.0) * bias1
    coef = np.empty((128, nblk, 4), np.float32)
    for blk in range(nblk):
        sl = slice(blk * 128, (blk + 1) * 128)
        coef[:, blk, 0] = inv1[sl]
        coef[:, blk, 1] = bias1z[sl]
        coef[:, blk, 2] = inv2[sl]
        coef[:, blk, 3] = bias2[sl]

    xr = np.ascontiguousarray(np.asarray(x).reshape(n, C, H * W),
                              dtype=np.float32)
    return [
        {"x": xr[i * n_img:(i + 1) * n_img], "wq": wq, "coef": coef}
        for i in range(_NCORES)
    ]


def _run(trace=False, **inputs):
    from concourse.bass_utils import run_bass_kernel_spmd

    n, C, H, W = inputs["x"].shape
    nc = _get_program(n // _NCORES, C, H, W, 8)
    in_maps = _make_in_maps(**inputs)
    res = run_bass_kernel_spmd(nc, in_maps, core_ids=list(range(_NCORES)),
                               trace=trace)
    out = np.concatenate([r["out"] for r in res.results], axis=0)
    return out.reshape(n, C, H, W), res


def kernel(x, w1, w2, g1, b1, m1, v1, g2, b2, m2, v2):
    out, _ = _run(x=x, w1=w1, w2=w2, g1=g1, b1=b1, m1=m1, v1=v1,
                  g2=g2, b2=b2, m2=m2, v2=v2)
    return out
